# revision 8
# baseline (speedup 1.0000x reference)
import sys
import numpy as np

sys.path.insert(0, "/opt/trn_rl_repo")

import concourse.bass as bass
import concourse.mybir as mybir
from concourse.bass_utils import run_bass_kernel_spmd

N_CORES = 8
C = 32
H = 64          # volume is 64^3
SLAB = H // N_CORES   # 8 planes per core along first spatial dim
PP = H + 2      # padded plane edge = 66
EPS = 1e-5


def _lrelu(x):
    return np.where(x >= 0, x, 0.2 * x).astype(np.float32)


def _build_conv_kernel(cin, cout):
    """Raw-Block bass program: 3x3x3 'valid' conv over a padded slab.

    Per-core input : x  [cin, SLAB+2, 66, 66]  (float32r, zero-padded halo)
                     w  [cin, 27*cout]         (tap-major lhsT)
    Per-core output: y  [cout, SLAB*64*64]     (float32)
    """
    nc = bass.Bass("TRN2", target_bir_lowering=False, debug=False,
                   num_devices=N_CORES)
    in_planes = SLAB + 2
    x_d = nc.declare_dram_parameter("x", [cin, in_planes * PP * PP],
                                    mybir.dt.float32r, isOutput=False)
    w_d = nc.declare_dram_parameter("w", [cin, 27 * cout],
                                    mybir.dt.float32r, isOutput=False)
    y_d = nc.declare_dram_parameter("y", [cout, SLAB * H * H],
                                    mybir.dt.float32, isOutput=True)

    ROWS_PER_TILE = 8            # 8 rows x 64 cols = 512 psum columns
    tiles = [(d, r) for d in range(SLAB) for r in range(H // ROWS_PER_TILE)]
    NT = len(tiles)              # 64

    with (
        nc.Block() as block,
        nc.semaphore("dma_sem") as dma_sem,
        nc.semaphore("mm_sem") as mm_sem,
        nc.semaphore("ev_sem") as ev_sem,
        nc.sbuf_tensor("x_sb", [cin, in_planes * PP * PP], mybir.dt.float32r) as x_sb,
        nc.sbuf_tensor("w_sb", [cin, 27 * cout], mybir.dt.float32r) as w_sb,
        nc.sbuf_tensor("y_sb", [cout, 4 * 512], mybir.dt.float32) as y_sb,
        nc.psum_tensor("acc", [cout, 8 * 512], mybir.dt.float32) as acc,
    ):
        x_v = x_sb.ap().rearrange("p (d h w) -> p d h w", d=in_planes, h=PP, w=PP)

        @block.sync
        def _(sync):
            sync.dma_start(out=x_sb[:, :], in_=x_d[:, :]).then_inc(dma_sem, 16)
            sync.dma_start(out=w_sb[:, :], in_=w_d[:, :]).then_inc(dma_sem, 16)
            for k in range(NT):
                sync.wait_ge(ev_sem, k + 1)
                sync.dma_start(
                    out=y_d[:, k * 512:(k + 1) * 512],
                    in_=y_sb[:, (k % 4) * 512:(k % 4) * 512 + 512],
                ).then_inc(dma_sem, 16)

        @block.tensor
        def _(tensor):
            tensor.wait_ge(dma_sem, 32)
            for k, (d, r) in enumerate(tiles):
                if k >= 8:
                    tensor.wait_ge(ev_sem, k - 7)
                bank = acc[:, (k % 8) * 512:(k % 8) * 512 + 512]
                t = 0
                for dz in range(3):
                    for dy in range(3):
                        for dx in range(3):
                            rhs = x_v[:, d + dz,
                                      r * ROWS_PER_TILE + dy:
                                      r * ROWS_PER_TILE + dy + ROWS_PER_TILE,
                                      dx:dx + H]
                            mm = tensor.matmul(
                                bank,
                                w_sb[:, t * cout:(t + 1) * cout],
                                rhs,
                                start=(t == 0), stop=(t == 26))
                            t += 1
                mm.then_inc(mm_sem, 1)

        @block.scalar
        def _(scalar):
            for k, (d, r) in enumerate(tiles):
                scalar.wait_ge(mm_sem, k + 1)
                if k >= 4:
                    # slot reuse: wait until tile k-4's out-DMA completed
                    scalar.wait_ge(dma_sem, 32 + 16 * (k - 3))
                dst = y_sb[:, (k % 4) * 512:(k % 4) * 512 + 512]
                scalar.copy(dst, acc[:, (k % 8) * 512:(k % 8) * 512 + 512]
                            ).then_inc(ev_sem, 1)

    return nc


def _run_conv(x_full_pad, w_lhsT, cin, cout):
    """x_full_pad: [cin, 66, 66, 66] (zero-padded volume). Returns [cout,64,64,64]."""
    nc = _build_conv_kernel(cin, cout)
    in_maps = []
    for k in range(N_CORES):
        sl = x_full_pad[:, k * SLAB:k * SLAB + SLAB + 2, :, :]
        in_maps.append({
            "x": np.ascontiguousarray(sl).reshape(cin, -1),
            "w": w_lhsT,
        })
    res = run_bass_kernel_spmd(nc, in_maps, list(range(N_CORES)))
    out = np.concatenate(
        [res.results[k]["y"].reshape(cout, SLAB, H, H) for k in range(N_CORES)],
        axis=1)
    return out


def _prep_w(w):
    """torch conv weight [O, I, 3,3,3] -> lhsT [I, 27*O] tap-major."""
    o, i = w.shape[0], w.shape[1]
    wt = w.reshape(o, i, 27).transpose(2, 1, 0)       # [27, I, O]
    return np.ascontiguousarray(wt.transpose(1, 0, 2).reshape(i, 27 * o)
                                ).astype(np.float32)


def _pad_vol(v):
    return np.pad(v, ((0, 0), (1, 1), (1, 1), (1, 1))).astype(np.float32)


def _inorm_lrelu(y):
    m = y.mean(axis=(1, 2, 3), keepdims=True, dtype=np.float64)
    va = y.var(axis=(1, 2, 3), keepdims=True, dtype=np.float64)
    z = ((y - m) / np.sqrt(va + EPS)).astype(np.float32)
    return _lrelu(z)


def _softmax(a, axis):
    m = a.max(axis=axis, keepdims=True)
    e = np.exp(a - m)
    return (e / e.sum(axis=axis, keepdims=True)).astype(np.float32)


def kernel(x_concat, w_cc1, b_cc1, w_cc2, b_cc2,
           w_down, b_down, g_down, be_down,
           w_adj1, b_adj1, g_adj1, be_adj1,
           w_adj2, b_adj2, g_adj2, be_adj2,
           w_adj3, b_adj3, gcn_w,
           w_up, b_up, g_up, be_up):
    x = np.asarray(x_concat, dtype=np.float32)[0]          # [32,64,64,64]

    # ---- conv_change_channels on device (2 launches, 8 cores each) ----
    h1 = _run_conv(_pad_vol(x), _prep_w(np.asarray(w_cc1)), C, 2 * C)
    h1 += np.asarray(b_cc1, np.float32)[:, None, None, None]
    h1n = _inorm_lrelu(h1)

    h2 = _run_conv(_pad_vol(h1n), _prep_w(np.asarray(w_cc2)), 2 * C, C)
    h2 += np.asarray(b_cc2, np.float32)[:, None, None, None]
    h = _inorm_lrelu(h2)                                    # [32,64,64,64]

    # ---- window partition: 16^3 windows of 4^3 ----
    WS = 4
    g_ = H // WS
    xw = h.reshape(C, g_, WS, g_, WS, g_, WS)
    xw = xw.transpose(1, 3, 5, 0, 2, 4, 6).reshape(g_ ** 3, C, WS, WS, WS)
    B = xw.shape[0]

    # conv_downsample k=2 s=2 + BN(eval) + lrelu
    wd = np.asarray(w_down, np.float32)
    xc = xw.reshape(B, C, 2, 2, 2, 2, 2, 2)                 # [B,C,i,dz,j,dy,k,dx]
    xd = np.einsum("bcizjykx,oczyx->boijk", xc, wd, optimize=True)
    xd += np.asarray(b_down, np.float32)[None, :, None, None, None]
    xd = xd * np.asarray(g_down, np.float32)[None, :, None, None, None] \
        + np.asarray(be_down, np.float32)[None, :, None, None, None]
    xd = _lrelu(xd)
    xn = xd.reshape(B, C, 8).transpose(0, 2, 1)             # [B,8,C]

    # adjacency MLP + masked softmax
    dif = np.abs(xn[:, :, None, :] - xn[:, None, :, :])     # [B,8,8,C]
    a1 = dif @ np.asarray(w_adj1, np.float32).T + np.asarray(b_adj1, np.float32)
    a1 = _lrelu(a1 * np.asarray(g_adj1, np.float32) + np.asarray(be_adj1, np.float32))
    a2 = a1 @ np.asarray(w_adj2, np.float32).T + np.asarray(b_adj2, np.float32)
    a2 = _lrelu(a2 * np.asarray(g_adj2, np.float32) + np.asarray(be_adj2, np.float32))
    s = a2 @ np.asarray(w_adj3, np.float32) + np.asarray(b_adj3, np.float32)[0]
    A = np.swapaxes(s, 1, 2)
    eye = np.eye(8, dtype=np.float32)
    P = _softmax(A - eye * 1e8, axis=2)

    # GCN: concat(eye@x, P@x) @ gcn_w, lrelu
    agg = np.concatenate([xn, P @ xn], axis=2)              # [B,8,2C]
    gout = _lrelu(agg @ np.asarray(gcn_w, np.float32))      # [B,8,C]

    # conv_upsample (ConvT k=2 s=2) + BN + lrelu
    xn2 = gout.transpose(0, 2, 1).reshape(B, C, 2, 2, 2)
    wu = np.asarray(w_up, np.float32)
    up = np.einsum("ncdhw,coijk->nodihjwk", xn2, wu, optimize=True)
    up = up.reshape(B, C, 4, 4, 4) + np.asarray(b_up, np.float32)[None, :, None, None, None]
    up = up * np.asarray(g_up, np.float32)[None, :, None, None, None] \
        + np.asarray(be_up, np.float32)[None, :, None, None, None]
    up = _lrelu(up)

    out = up.reshape(1, g_, g_, g_, C, WS, WS, WS)
    out = out.transpose(0, 4, 1, 5, 2, 6, 3, 7).reshape(1, C, H, H, H)
    return out.astype(np.float32)
